# revision 1
# baseline (speedup 1.0000x reference)
"""Multi-Head Latent Attention (MLA) Trainium2 kernel, v2.

Problem: B=4, T=2048, C=768, H=12, D=64, R=64, causal attention, RoPE.
Sharding: 8 cores = 4 batches x 2 head-groups (6 heads each). Each core
computes a partial output y_partial[b] (f16); host sums the two head-group
partials per batch and adds the bias.

v2 structural changes vs baseline:
- jq-outer attention loop; softmax normalization (B2) runs per (pair, jq)
  via nc.vector.reciprocal on the PSUM denominator row (no cross-partition
  DMA shuffle), and the output projection (C) is interleaved per 1024-token
  chunk so PE-heavy C overlaps ACT-bound attention of the next chunk.
- LayerNorm: mean comes from an extra host-built Wdown column; ln_g is
  folded into Wup on the host; ln_b is folded into the host-side output
  bias (v path) and an optional on-chip rope-rotated k-bias track (only
  built when ln_b != 0, which never happens with this reference).
- y output in f16 (halves the output transfer), cos/sin shipped as [64,T].
- Engine balance: exp() owns ACT in phase B; evictions are placed on
  whichever engine has slack in that window.
"""
import numpy as np
import ml_dtypes

import jax
from jax.sharding import Mesh, PartitionSpec
from jax.experimental.shard_map import shard_map

import concourse.bass as bass
import concourse.mybir as mybir
import concourse.tile as tile
from concourse.bass2jax import (_bass_exec_p, install_neuronx_cc_hook,
                                partition_id_tensor)
from concourse.masks import make_identity

BF16 = mybir.dt.bfloat16
F16 = mybir.dt.float16
F32 = mybir.dt.float32

B, T, C = 4, 2048, 768
H, D = 12, 64
R = 64
HL = 6              # heads per core
NP = HL // 2        # head pairs per core
ROPE_THETA = 10000.0
N_CORES = 8

TT = T // 128       # 16 token tiles
CC = C // 128       # 6 contraction chunks
QC = T // 512       # 4 proj chunks of 512

_cached = {}


def _split_sync_waits(nc, max_waits=1):
    """Split instructions carrying >1 sem wait into wait-carrier NoOps
    (this walrus build supports a single sync wait per instruction)."""
    for f in nc.m.functions:
        for bb in f.blocks:
            new_list, changed = [], False
            for ins in bb.instructions:
                si = ins.sync_info
                waits = list(si.on_wait) if si is not None else []
                if len(waits) > max_waits:
                    excess, keep = waits[:-max_waits], waits[-max_waits:]
                    for i in range(0, len(excess), max_waits):
                        nop = mybir.InstNoOp(
                            name=f"waitsplit-{nc.next_id()}",
                            engine=ins.engine, ins=[], outs=[],
                            sync_info=mybir.SyncInfo(
                                on_wait=excess[i:i + max_waits], on_update=[]))
                        nc.register_instruction(nop)
                        new_list.append(nop)
                    ins.sync_info = mybir.SyncInfo(
                        on_wait=keep, on_update=list(si.on_update))
                    changed = True
                new_list.append(ins)
            if changed:
                bb.instructions = new_list


def _build_nc(has_lnb=False):
    nc = bass.Bass("TRN2", target_bir_lowering=False)

    # ---- DRAM I/O ----
    xT_d = nc.dram_tensor("xT", [C, T], BF16, kind="ExternalInput")
    wq_d = nc.dram_tensor("wq", [C, HL * D], BF16, kind="ExternalInput")
    # wdown has an extra mean column (col R = Wdown.mean(axis=1))
    wdown_d = nc.dram_tensor("wdownx", [C, R + 1], BF16, kind="ExternalInput")
    # up-proj weights padded to K=128 on the host (rows 64..127 zero), ln_g
    # folded in
    wupk_d = nc.dram_tensor("wupk", [128, HL * D], BF16, kind="ExternalInput")
    wupkr_d = nc.dram_tensor("wupkr", [128, HL * D], BF16,
                             kind="ExternalInput")
    wupv_d = nc.dram_tensor("wupv", [128, HL * D], BF16, kind="ExternalInput")
    wo_d = nc.dram_tensor("wo", [HL * D, C], BF16, kind="ExternalInput")
    cos_d = nc.dram_tensor("cos64", [64, T], BF16, kind="ExternalInput")
    sin_d = nc.dram_tensor("sin64", [64, T], BF16, kind="ExternalInput")
    mask_d = nc.dram_tensor("mask128", [128, 128], BF16, kind="ExternalInput")
    if has_lnb:
        # rope-rotated k-bias track per pair: kT += rbk after rope
        rbk_d = nc.dram_tensor("rbk", [128, NP, T], BF16,
                               kind="ExternalInput")
    y_d = nc.dram_tensor("y", [T, C], F16, kind="ExternalOutput")

    AX = mybir.AxisListType.X
    MUL = mybir.AluOpType.mult
    ADD = mybir.AluOpType.add
    SUB = mybir.AluOpType.subtract

    with tile.TileContext(nc) as tc:
        with tc.tile_pool(name="persist", bufs=1) as pp, \
             tc.tile_pool(name="psmm", bufs=2, space="PSUM") as psmm, \
             tc.tile_pool(name="psoa", bufs=2, space="PSUM") as psoa, \
             tc.tile_pool(name="work", bufs=3) as work:
            # ---------------- persistent tiles ----------------
            xT_t = pp.tile([128, CC, T], BF16)
            wq_t = pp.tile([128, CC, HL * D], BF16)
            wdown_t = pp.tile([128, CC, R + 1], BF16)
            wupk_t = pp.tile([128, HL * D], BF16)
            wupkr_t = pp.tile([128, HL * D], BF16)
            wupv_t = pp.tile([128, HL * D], BF16)
            wo2_t = pp.tile([128, NP, C], BF16)
            cos_t = pp.tile([128, T], BF16)
            sin_t = pp.tile([128, T], BF16)
            mask_t = pp.tile([128, 128], BF16)
            ident_t = pp.tile([128, 128], BF16)
            ckvT_t = pp.tile([128, T], BF16)      # rows 64..127 zero (K pad)
            ckv_sb = pp.tile([128, TT, R + 1], BF16)
            mu_t = pp.tile([128, TT], F32)
            s2_t = pp.tile([128, TT], F32)
            var_t = pp.tile([128, TT], F32)
            rstd_t = pp.tile([128, TT], F32)
            mus_t = pp.tile([128, TT], F32)
            ones_t = pp.tile([128, 64], BF16)
            if has_lnb:
                rbk_t = pp.tile([128, NP, T], BF16)

            qT_p = [pp.tile([128, T], BF16, name=f"qT{p}") for p in range(NP)]
            kT_p = [pp.tile([128, T], BF16, name=f"kT{p}") for p in range(NP)]
            # V pair: [.., 0:64]=V_even, 64=ones, [.., 65:129]=V_odd, 129=ones
            v_p = [pp.tile([128, TT, 130], BF16, name=f"v{p}")
                   for p in range(NP)]

            # ---------------- input DMAs ----------------
            # inputs spread across the three DMA queues: x on SP,
            # weights on the gpsimd SWDGE, rope tables on the ACT HWDGE
            wdown_r = wdown_d.rearrange("(a p) n -> p a n", p=128)
            nc.gpsimd.dma_start(wdown_t[:, :, :], wdown_r)
            nc.gpsimd.dma_start(wq_t[:, :, :],
                                wq_d.rearrange("(a p) n -> p a n", p=128))
            xT_r = xT_d.rearrange("(a p) n -> p a n", p=128)
            for qq in range(4):
                qs = slice(qq * 512, (qq + 1) * 512)
                nc.sync.dma_start(xT_t[:, :, qs], xT_r[:, :, qs])
            nc.gpsimd.dma_start(wupk_t[:, :], wupk_d[:, :])
            nc.gpsimd.dma_start(wupkr_t[:, :], wupkr_d[:, :])
            nc.gpsimd.dma_start(wupv_t[:, :], wupv_d[:, :])
            nc.scalar.dma_start(cos_t[0:64, :], cos_d[:, :])
            nc.scalar.dma_start(cos_t[64:128, :], cos_d[:, :])
            nc.scalar.dma_start(sin_t[0:64, :], sin_d[:, :])
            nc.scalar.dma_start(sin_t[64:128, :], sin_d[:, :])
            nc.gpsimd.dma_start(mask_t[:, :], mask_d[:, :])
            for p in range(NP):
                nc.gpsimd.dma_start(wo2_t[:, p, :],
                                    wo_d[p * 128:(p + 1) * 128, :])
            if has_lnb:
                for p in range(NP):
                    nc.gpsimd.dma_start(rbk_t[:, p, :], rbk_d[:, p, :])
            make_identity(nc, ident_t[:, :])
            nc.vector.memset(ones_t[:, :], 1.0)
            for p in range(NP):
                nc.vector.memset(v_p[p][:, :, D:D + 1], 1.0)
                nc.vector.memset(v_p[p][:, :, 2 * D + 1:2 * D + 2], 1.0)

            # ---------------- Phase A: ckv + LN ----------------
            # fully pipelined per 4-tile group so ckvT (and with it the
            # k/v projections) is available as soon as each x chunk lands
            for g in range(4):
                gt = slice(4 * g, 4 * (g + 1))
                ps_c = psmm.tile([128, 4, R + 1], F32, tag="mm")
                for tt in range(4):
                    it = 4 * g + tt
                    ts_ = slice(it * 128, (it + 1) * 128)
                    for cc in range(CC):
                        nc.tensor.matmul(
                            ps_c[:, tt, :], xT_t[:, cc, ts_],
                            wdown_t[:, cc, :],
                            start=(cc == 0), stop=(cc == CC - 1))
                nc.scalar.copy(ckv_sb[:, gt, :], ps_c[:, :, :])

                sqv = work.tile([128, 4, R], BF16, tag="sqv")
                nc.vector.tensor_mul(sqv[:, :, :], ckv_sb[:, gt, 0:R],
                                     ckv_sb[:, gt, 0:R])
                nc.vector.reduce_sum(s2_t[:, gt], sqv[:, :, :], axis=AX)
                nc.vector.tensor_copy(mu_t[:, gt], ckv_sb[:, gt, R])
                musq = work.tile([128, 4], F32, tag="musq")
                nc.vector.tensor_mul(musq[:, :], mu_t[:, gt], mu_t[:, gt])
                # var = s2/R - mu^2 + eps  (E[x^2] - E[x]^2)
                nc.vector.tensor_scalar(var_t[:, gt], s2_t[:, gt],
                                        1.0 / R, 1e-5, op0=MUL, op1=ADD)
                nc.vector.tensor_sub(var_t[:, gt], var_t[:, gt], musq[:, :])
                # rstd = exp(-0.5*ln(var+eps))
                nc.scalar.activation(var_t[:, gt], var_t[:, gt],
                                     mybir.ActivationFunctionType.Ln)
                nc.scalar.activation(rstd_t[:, gt], var_t[:, gt],
                                     mybir.ActivationFunctionType.Exp,
                                     scale=-0.5)
                nc.vector.tensor_mul(mus_t[:, gt], mu_t[:, gt],
                                     rstd_t[:, gt])
                for tt in range(4):
                    it = 4 * g + tt
                    # in-place standardize: x*rstd - mu*rstd
                    nc.vector.tensor_scalar(
                        ckv_sb[:, it, 0:R], ckv_sb[:, it, 0:R],
                        rstd_t[:, it:it + 1], mus_t[:, it:it + 1],
                        op0=MUL, op1=SUB)
                ps_t = psmm.tile([R, 4, 128], BF16, tag="mm")
                for tt in range(4):
                    it = 4 * g + tt
                    nc.tensor.transpose(ps_t[:, tt, :], ckv_sb[:, it, 0:R],
                                        ident_t[:, :])
                gs = slice(g * 512, (g + 1) * 512)
                nc.vector.tensor_copy(
                    ckvT_t[0:R, gs].rearrange("a (t n) -> a t n", t=4),
                    ps_t[:, :, :])
                nc.vector.tensor_copy(
                    ckvT_t[R:2 * R, gs].rearrange("a (t n) -> a t n", t=4),
                    ps_t[:, :, :])

            def _emit_s_block(p, q0, i):
                # both pair members stacked in one [128, 2, 512] tile per
                # 512-chunk -> one exp covers both
                qlo = max(128 * i, q0)
                width = q0 + 1024 - qlo
                kt = slice(128 * i, 128 * (i + 1))
                chunks = []
                off = 0
                while off < width:
                    w = min(512, width - off)
                    ps_s2 = psmm.tile([128, 2, 512], F32, tag="mm")
                    for m in range(2):
                        hp = slice(64 * m, 64 * (m + 1))
                        nc.tensor.matmul(
                            ps_s2[:, m, 0:w],
                            kT_p[p][hp, kt],
                            qT_p[p][hp, qlo + off:qlo + off + w],
                            start=True, stop=True)
                    chunks.append((off, w, ps_s2))
                    off += w
                return chunks

            # ---------------- Phase A: q/k/v per pair ----------------
            def _emit_a_pair(p, pre_b=None):
                pc = slice(p * 128, (p + 1) * 128)
                # q projections with lag-1 rotation matmuls so PE never
                # stalls on the ACT eviction of the chunk it just produced
                qq_tiles = [None] * QC
                sq_tiles = [None] * QC

                def _emit_qmm(jc):
                    qs = slice(jc * 512, (jc + 1) * 512)
                    ps_qq = psmm.tile([128, 512], F32, tag="mm")
                    for cc in range(CC):
                        nc.tensor.matmul(
                            ps_qq[:, 0:512], wq_t[:, cc, pc], xT_t[:, cc, qs],
                            start=(cc == 0), stop=(cc == CC - 1))
                    sq_ = work.tile([128, 512], BF16, tag="sq_", bufs=4)
                    nc.scalar.copy(sq_[:, :], ps_qq[:, 0:512])
                    qq_tiles[jc] = ps_qq
                    sq_tiles[jc] = sq_

                def _emit_qrot(jc):
                    qs = slice(jc * 512, (jc + 1) * 512)
                    sq_ = sq_tiles[jc]
                    # rotate-half via 32-aligned partition-shifted DVE ops
                    sqr = work.tile([128, 512], BF16, tag="sqr", bufs=4)
                    for hh in range(2):
                        b = 64 * hh
                        nc.vector.tensor_scalar_mul(
                            sqr[b:b + 32, :], sq_[b + 32:b + 64, :], -1.0)
                        nc.vector.tensor_copy(
                            sqr[b + 32:b + 64, :], sq_[b:b + 32, :])
                    t1 = work.tile([128, 512], BF16, tag="t1", bufs=4)
                    t2 = work.tile([128, 512], BF16, tag="t2", bufs=4)
                    nc.gpsimd.tensor_mul(t1[:, :], sq_[:, :], cos_t[:, qs])
                    nc.vector.tensor_mul(t2[:, :], sqr[:, :], sin_t[:, qs])
                    nc.gpsimd.tensor_add(qT_p[p][:, qs], t1[:, :], t2[:, :])

                _emit_qmm(0)
                for jc in range(1, QC):
                    _emit_qmm(jc)
                    _emit_qrot(jc - 1)
                _emit_qrot(QC - 1)

                # k/kr: one combined PSUM tile per chunk, single ACT
                # eviction, all-SBUF rope on DVE fast modes
                for jc in range(QC):
                    qs = slice(jc * 512, (jc + 1) * 512)
                    ps_kk = psmm.tile([128, 1024], F32, tag="mm")
                    nc.tensor.matmul(ps_kk[:, 0:512], wupk_t[0:64, pc],
                                     ckvT_t[0:64, qs], start=True, stop=True)
                    nc.tensor.matmul(ps_kk[:, 512:1024], wupkr_t[64:128, pc],
                                     ckvT_t[64:128, qs],
                                     start=True, stop=True)
                    skk = work.tile([128, 1024], BF16, tag="skk", bufs=4)
                    nc.scalar.copy(skk[:, :], ps_kk[:, :])
                    t3 = work.tile([128, 512], BF16, tag="t1", bufs=4)
                    t4 = work.tile([128, 512], BF16, tag="t2", bufs=4)
                    nc.vector.tensor_mul(t3[:, :], skk[:, 0:512],
                                         cos_t[:, qs])
                    nc.vector.tensor_mul(t4[:, :], skk[:, 512:1024],
                                         sin_t[:, qs])
                    if has_lnb:
                        nc.vector.tensor_add(t3[:, :], t3[:, :],
                                             rbk_t[:, p, qs])
                    nc.vector.tensor_add(kT_p[p][:, qs], t3[:, :], t4[:, :])

                pre = pre_b() if pre_b is not None else None
                for g in range(4):
                    ps_v = psmm.tile([128, 4, 128], F32, tag="mm")
                    for tt in range(4):
                        it = 4 * g + tt
                        ts_ = slice(it * 128, (it + 1) * 128)
                        nc.tensor.matmul(
                            ps_v[:, tt, :], ckvT_t[0:64, ts_],
                            wupv_t[0:64, pc], start=True, stop=True)
                    vv = v_p[p][:, 4 * g:4 * (g + 1), :].rearrange(
                        "a t (g c) -> a t g c", g=2)
                    nc.vector.tensor_copy(
                        vv[:, :, :, 0:D],
                        ps_v[:, :, :].rearrange("a t (g c) -> a t g c", g=2))
                return pre

            # ---------------- Phase B/C: jq-outer attention ----------------
            def _make_b2(p, jq, ps_oe, ps_oo, cs=((), ())):
                """Normalization for (pair, jq): recip of the denominator
                rows (kept at 32-aligned partitions), ones-matmul broadcast,
                divide folded into the eviction multiply."""
                q0 = jq * 1024

                def emit():
                    dnr = work.tile([128, 1024], BF16, tag="dnr")
                    with nc.allow_low_precision("softmax denom recip bf16"):
                        nc.vector.reciprocal(dnr[64:65, :],
                                             ps_oe[D:D + 1, :])
                        nc.vector.reciprocal(dnr[32:33, :],
                                             ps_oo[D:D + 1, :])
                    rb_sb = work.tile([128, 1024], BF16, tag="rbsb")
                    for h in range(2):
                        hs = slice(h * 512, (h + 1) * 512)
                        ps_rb = psmm.tile([128, 512], F32, tag="mm")
                        nc.tensor.matmul(ps_rb[0:64, :],
                                         ones_t[64:65, 0:64], dnr[64:65, hs],
                                         start=True, stop=True)
                        nc.tensor.matmul(ps_rb[64:128, :],
                                         ones_t[32:33, 0:64], dnr[32:33, hs],
                                         start=True, stop=True)
                        nc.vector.tensor_copy(rb_sb[:, hs], ps_rb[:, :])
                    for m, ps_o in ((0, ps_oe), (1, ps_oo)):
                        hp2 = slice(64 * m, 64 * (m + 1))
                        nc.vector.tensor_mul(
                            qT_p[p][hp2, q0:q0 + 1024],
                            ps_o[0:D, :], rb_sb[hp2, :])
                    for h in range(2):
                        for c in cs[h]:
                            c()
                return emit

            def _make_c_tile(it, on_act):
                ts_ = slice(it * 128, (it + 1) * 128)

                def emit():
                    y_sb = work.tile([128, C], F16, tag="ysb", bufs=2)
                    dma_eng = nc.gpsimd if it % 2 == 0 else nc.sync
                    for half in range(2):
                        ns = slice(half * 384, (half + 1) * 384)
                        ps_y = psmm.tile([128, 384], F32, tag="mm")
                        for p in range(NP):
                            nc.tensor.matmul(
                                ps_y[:, :], qT_p[p][:, ts_],
                                wo2_t[:, p, ns],
                                start=(p == 0), stop=(p == NP - 1))
                        if on_act:
                            nc.scalar.copy(y_sb[:, ns], ps_y[:, :])
                        else:
                            nc.vector.tensor_copy(y_sb[:, ns], ps_y[:, :])
                    dma_eng.dma_start(y_d[ts_, :], y_sb[:, :])
                return emit

            pending_c = []

            def _emit_b_pair(jq, p, pending_b2, cs=((), ()),
                             split_last=False, s0=None):
                q0 = jq * 1024
                last_i = 8 * jq + 7
                if True:
                    ps_oe = psoa.tile([D + 1, 1024], F32, tag="o")
                    ps_oo = psoa.tile([D + 1, 1024], F32, tag="o")

                    def _emit_s(i):
                        return _emit_s_block(p, q0, i)

                    # prologue: S of block 0 (possibly pre-issued during
                    # this pair's v projection)
                    s_chunks = s0 if s0 is not None else _emit_s(0)
                    # previous pair's normalization lands here, after this
                    # pair's first S so ACT has exp work during it
                    if pending_b2 is not None:
                        pending_b2()
                        pending_b2 = None

                    for i in range(last_i + 1):
                        qlo = max(128 * i, q0)
                        width = q0 + 1024 - qlo
                        pT_chunks = []
                        for off, w, ps_s2 in s_chunks:
                            pT = work.tile([128, 2, 512], BF16, tag="pT",
                                           bufs=8)
                            nc.scalar.activation(
                                pT[:, :, 0:w], ps_s2[:, :, 0:w],
                                mybir.ActivationFunctionType.Exp,
                                scale=float(D) ** -0.5)
                            if off == 0 and 128 * i >= q0:
                                for m in range(2):
                                    nc.gpsimd.tensor_mul(
                                        pT[:, m, 0:128], pT[:, m, 0:128],
                                        mask_t[:, :])
                            pT_chunks.append((off, w, pT))
                        # issue-ahead: next block's S before this block's AV
                        # so PE has work while ACT runs exp
                        if i < last_i:
                            s_chunks = _emit_s(i + 1)
                        h0_last = (8 * jq + 3) if split_last else last_i
                        for m in range(2):
                            ps_o = ps_oe if m == 0 else ps_oo
                            vsl = (slice(0, D + 1) if m == 0
                                   else slice(D + 1, 2 * D + 2))
                            for off, w, pT in pT_chunks:
                                o2 = 0
                                while o2 < w:
                                    pos = qlo - q0 + off + o2
                                    ww = min(512 - (pos % 512), w - o2)
                                    stop_i = (h0_last if pos < 512
                                              else last_i)
                                    nc.tensor.matmul(
                                        ps_o[:, pos:pos + ww],
                                        v_p[p][:, i, vsl],
                                        pT[:, m, o2:o2 + ww],
                                        start=(i == 0), stop=(i == stop_i),
                                        skip_group_check=True)
                                    o2 += ww
                        # previous jq's output-projection tiles ride in the
                        # AV->S slot gaps of pair 0
                        if p == 0 and pending_c:
                            pending_c.pop(0)()

                return _make_b2(p, jq, ps_oe, ps_oo, cs=cs)

            # drive: each pair's projections followed immediately by its
            # jq0 attention, so exp work exists during the next pair's
            # projections; then jq1; C(jq0) rides inside jq1 pair 0.
            pending_b2 = None
            for p in range(NP):
                s0 = _emit_a_pair(
                    p, pre_b=lambda pp=p: _emit_s_block(pp, 0, 0))
                pending_b2 = _emit_b_pair(0, p, pending_b2, s0=s0)
            # pre-issue jq1 pair-0's first S so exp bridges the transition
            s0_jq1 = _emit_s_block(0, 1024, 0)
            pending_c.extend(
                _make_c_tile(itl, on_act=False) for itl in range(8))
            c1 = [_make_c_tile(8 + itl, on_act=True) for itl in range(8)]
            for p in range(NP):
                last = p == NP - 1
                pending_b2 = _emit_b_pair(
                    1, p, pending_b2,
                    cs=(c1[0:4], c1[4:8]) if last else ((), ()),
                    s0=s0_jq1 if p == 0 else None)
            pending_b2()

    _split_sync_waits(nc)
    return nc


def _host_inputs(x, Wq, Wdown, ln_g, ln_b, Wup, Wo):
    """Prepare the 8 per-core input maps (host-side sharding)."""
    bf = ml_dtypes.bfloat16
    inv_freq = 1.0 / (ROPE_THETA ** (np.arange(0, D, 2, dtype=np.float64) / D))
    ang = np.arange(T, dtype=np.float64)[None, :] * inv_freq[:, None]  # [D/2,T]
    ang = np.concatenate([ang, ang], axis=0)                            # [D, T]
    cos64 = np.cos(ang).astype(np.float32)
    sin64 = np.sin(ang).astype(np.float32)

    d2 = D // 2
    perm = np.concatenate([np.arange(d2, D), np.arange(0, d2)])
    sign = np.concatenate([-np.ones(d2), np.ones(d2)]).astype(np.float32)

    def rotcols(W):
        Wr = W.reshape(W.shape[0], -1, D)
        Wr = Wr[:, :, perm] * sign[None, None, :]
        return Wr.reshape(W.shape)

    def padk(W):  # [64, N] -> [128, N] zero-padded
        return np.concatenate([W, np.zeros_like(W)], axis=0)

    def padk_hi(W):  # [64, N] -> [128, N] with data in rows 64..127
        return np.concatenate([np.zeros_like(W), W], axis=0)

    gWup = Wup * ln_g[:, None]
    Wup_k = gWup[:, 0:H * D]
    Wup_v = gWup[:, H * D:2 * H * D]
    Wup_k_rot = rotcols(Wup_k)

    # wdown with mean column
    wdownx = np.concatenate([Wdown, Wdown.mean(axis=1, keepdims=True)],
                            axis=1)

    mask128 = (np.arange(128)[None, :] >= np.arange(128)[:, None])

    has_lnb = bool(np.any(ln_b != 0.0))
    bk = ln_b @ Wup[:, 0:H * D]          # [H*D] k-bias (un-g-scaled: b@Wup)
    if has_lnb:
        # rope-rotated per-head k-bias track rbk[d, t] for each head
        bk_h = bk.reshape(H, D)
        rot_bk = bk_h[:, perm] * sign[None, :]
        # [H, D, T]
        rbk_full = (cos64[None, :, :] * bk_h[:, :, None]
                    + sin64[None, :, :] * rot_bk[:, :, None])

    in_maps = []
    for core in range(N_CORES):
        b = core // 2
        hg = core % 2
        hs = slice(hg * HL * D, (hg + 1) * HL * D)
        m = {
            "xT": np.ascontiguousarray(x[b].T).astype(bf),
            "wq": Wq[:, hs].astype(bf),
            "wdownx": wdownx.astype(bf),
            "wupk": padk(Wup_k[:, hs].astype(bf)),
            "wupkr": padk_hi(Wup_k_rot[:, hs].astype(bf)),
            "wupv": padk(Wup_v[:, hs].astype(bf)),
            "wo": Wo[hs, :].astype(bf),
            "cos64": cos64.astype(bf),
            "sin64": sin64.astype(bf),
            "mask128": mask128.astype(np.float32).astype(bf),
        }
        if has_lnb:
            h0 = hg * HL
            rbk = np.zeros((128, NP, T), np.float32)
            for p in range(NP):
                rbk[0:D, p, :] = rbk_full[h0 + 2 * p]
                rbk[D:128, p, :] = rbk_full[h0 + 2 * p + 1]
            m["rbk"] = rbk.astype(bf)
        in_maps.append(m)
    return in_maps


def _get_runner(has_lnb=False):
    """Build the bass program once and a cached jitted 8-core executor."""
    key = ("runner", has_lnb)
    if key in _cached:
        return _cached[key]
    install_neuronx_cc_hook()
    nc = _build_nc(has_lnb)
    partition_name = (nc.partition_id_tensor.name
                      if nc.partition_id_tensor else None)
    in_names, out_names, out_avals, zero_outs = [], [], [], []
    for alloc in nc.m.functions[0].allocations:
        if not isinstance(alloc, mybir.MemoryLocationSet):
            continue
        name = alloc.memorylocations[0].name
        if alloc.kind == "ExternalInput":
            if name != partition_name:
                in_names.append(name)
        elif alloc.kind == "ExternalOutput":
            out_names.append(name)
            shape = tuple(alloc.tensor_shape)
            dtype = mybir.dt.np(alloc.dtype)
            out_avals.append(jax.core.ShapedArray(shape, dtype))
            zero_outs.append(np.zeros(shape, dtype))
    n_params = len(in_names)
    all_in_names = list(in_names) + list(out_names)
    if partition_name is not None:
        all_in_names.append(partition_name)

    def _body(*args):
        operands = list(args)
        if partition_name is not None:
            operands.append(partition_id_tensor())
        return tuple(_bass_exec_p.bind(
            *operands,
            out_avals=tuple(out_avals),
            in_names=tuple(all_in_names),
            out_names=tuple(out_names),
            lowering_input_output_aliases=(),
            sim_require_finite=True,
            sim_require_nnan=True,
            nc=nc,
        ))

    devices = jax.devices()[:N_CORES]
    mesh = Mesh(np.asarray(devices), ("core",))
    in_specs = (PartitionSpec("core"),) * (n_params + len(out_names))
    out_specs = (PartitionSpec("core"),) * len(out_names)
    fn = jax.jit(shard_map(_body, mesh=mesh, in_specs=in_specs,
                           out_specs=out_specs, check_rep=False),
                 keep_unused=True)

    def run(in_maps):
        concat_in = [np.concatenate([np.asarray(in_maps[c][nm])
                                     for c in range(N_CORES)], axis=0)
                     for nm in in_names]
        concat_zeros = [np.zeros((N_CORES * z.shape[0], *z.shape[1:]), z.dtype)
                        for z in zero_outs]
        out_arrs = fn(*concat_in, *concat_zeros)
        return [{name: np.asarray(out_arrs[i]).reshape(
                    N_CORES, *out_avals[i].shape)[c]
                 for i, name in enumerate(out_names)}
                for c in range(N_CORES)]

    _cached[key] = run
    return run


def kernel(x, Wq, Wdown, ln_g, ln_b, Wup, Wo, bo):
    x = np.asarray(x, dtype=np.float32)
    Wq = np.asarray(Wq, dtype=np.float32)
    Wdown = np.asarray(Wdown, dtype=np.float32)
    ln_g = np.asarray(ln_g, dtype=np.float32)
    ln_b = np.asarray(ln_b, dtype=np.float32)
    Wup = np.asarray(Wup, dtype=np.float32)
    Wo = np.asarray(Wo, dtype=np.float32)
    bo = np.asarray(bo, dtype=np.float32)

    has_lnb = bool(np.any(ln_b != 0.0))
    run = _get_runner(has_lnb)
    in_maps = _host_inputs(x, Wq, Wdown, ln_g, ln_b, Wup, Wo)
    results = run(in_maps)

    # host-side bias: bo plus the v-path ln_b contribution
    bv = ln_b @ Wup[:, H * D:2 * H * D]        # [H*D]
    y_bias = bo + bv @ Wo                      # [C]

    out = np.empty((B, T, C), dtype=np.float32)
    for b in range(B):
        out[b] = (results[2 * b]["y"].astype(np.float32)
                  + results[2 * b + 1]["y"].astype(np.float32)
                  + y_bias[None, :])
    return out

